# revision 3
# baseline (speedup 1.0000x reference)
"""Trainium2 Bass kernel for nn_AttractorState — sequence-parallel, tuned.

See kernel_v3.py for the math/sharding derivation.  Differences vs v3:

- 2-tile DMA chunks (0.5 MiB/transfer): halves the DMA->DVE->PE dependency
  latency while keeping per-transfer efficiency; queues stay saturated via
  8-deep buffer rings.
- W^T is fetched AFTER the last input chunk is issued: it is only consumed
  at the very end (Y = W @ G), and fetching it early steals ~2.5 us of
  front bandwidth from the h/pe stream that paces the whole kernel.
- Tail: Y matmuls run cs-major; each Y row-block is copied out right after
  its last matmul (alternating DVE/ACT), and the output DMA is split in
  two so it overlaps the remaining copies.
- gred (G PSUM -> SBUF bf16) copies alternate DVE/ACT.

Empirical model this is tuned against (from ntff traces): PE streams bf16
matmul columns at ~1.2 GHz while the DMA engines are busy (hardware
activity throttling) and ~2.4 GHz when they are not, so the kernel is
DMA-window bound (~41 us for 16.5 MiB) plus a PE catch-up tail (~10 us)
plus startup (~11 us) — every us of dependency slack shows up 1:1 in the
total.
"""

import math
import sys

import numpy as np

for _p in ("/opt/trn_rl_repo", "/opt/trn_rl_repo/concourse"):
    if _p not in sys.path:
        sys.path.append(_p)

# Problem constants (hardcoded per harness contract).
B = 4
S = 8192
D = 512          # d_model
E = 512          # d_state
P = 128          # SBUF partitions
NCORES = 8
SS = S // 2      # 4096 tokens per core
NT = SS // P     # 32 t-tiles per core
SIZES = [1, 1] + [2] * 14 + [1, 1]
assert sum(SIZES) == NT
CH = len(SIZES)
TPC = max(SIZES)  # buffer sizing

_GRAPH_CACHE = {}


def _decay_weights():
    # Match reference: alpha = f32(exp(-pi/S)); w = exp((S-1-t) * log(alpha)) in f32.
    alpha = np.float32(math.exp(-math.pi / S))
    t = np.arange(S, dtype=np.float32)
    w = np.exp((np.float32(S - 1.0) - t) * np.log(alpha)).astype(np.float32)
    return w


def _build(bias: bool):
    key = ("bias" if bias else "nobias")
    if key in _GRAPH_CACHE:
        return _GRAPH_CACHE[key]

    import concourse.bass as bass  # noqa: F401
    import concourse.mybir as mybir
    import concourse.tile as tile
    from concourse import bacc

    f32 = mybir.dt.float32
    bf16 = mybir.dt.bfloat16
    AF = mybir.ActivationFunctionType

    nc = bacc.Bacc("TRN2", target_bir_lowering=False)

    h_ext = nc.declare_dram_parameter("h", [SS, D], f32, isOutput=False)
    pe_ext = nc.declare_dram_parameter("pe", [SS, D], f32, isOutput=False)
    # W^T staged bf16 host-side: the kernel computes in bf16 anyway, so the
    # rounding is inherent; this halves the wt transfer and removes the
    # on-chip cast from the tail's critical path.
    wt_ext = nc.declare_dram_parameter("wt", [D, E], bf16, isOutput=False)
    b_ext = nc.declare_dram_parameter("b", [E], f32, isOutput=False)
    wdec_ext = nc.declare_dram_parameter("wdec", [P, NT], f32, isOutput=False)
    out_ext = nc.declare_dram_parameter("out", [E, D], f32, isOutput=True)

    h_re = h_ext.ap().rearrange("(n p) d -> p n d", p=P)
    pe_re = pe_ext.ap().rearrange("(n p) d -> p n d", p=P)
    wt_re = wt_ext.ap().rearrange("(c p) e -> p c e", p=P)
    out_re = out_ext.ap().rearrange("(c p) d -> p c d", p=P)

    with tile.TileContext(nc) as tc:
        with (
            tc.tile_pool(name="consts", bufs=1) as consts,
            tc.tile_pool(name="io", bufs=8) as io,
            tc.tile_pool(name="acc", bufs=1, space="PSUM") as acc_pool,
            tc.tile_pool(name="pst", bufs=2, space="PSUM") as pst,
        ):
            # ---- input stream leads everything ----
            # (wdec rides the otherwise-idle gpsimd SWDGE)
            wdec_sb = consts.tile([P, NT], f32)
            nc.gpsimd.dma_start(wdec_sb[:], wdec_ext[:, :])
            starts = [sum(SIZES[:j]) for j in range(CH)]
            h_tiles = {}
            pe_tiles = {}
            PRE = 6
            for j in range(PRE):
                n0, w = starts[j], SIZES[j]
                h_t = io.tile([P, TPC, D], f32, tag="h", name=f"h_t{j}")
                pe_t = io.tile([P, TPC, D], f32, tag="pe", name=f"pe_t{j}")
                nc.sync.dma_start(h_t[:, 0:w, :], h_re[:, n0:n0 + w, :])
                nc.scalar.dma_start(pe_t[:, 0:w, :], pe_re[:, n0:n0 + w, :])
                h_tiles[j] = h_t
                pe_tiles[j] = pe_t

            if bias:
                wdec_bf = consts.tile([P, NT], bf16)
                nc.vector.tensor_copy(wdec_bf[:], wdec_sb[:])
                b_sb = consts.tile([1, E], f32)
                nc.gpsimd.dma_start(b_sb[:], b_ext.ap().unsqueeze(0))
                b_bf = consts.tile([1, E], bf16)
                nc.vector.tensor_copy(b_bf[:], b_sb[:])

            # ---- G (/ r) accumulation over this shard's 4096 tokens ----
            g_ps = [
                acc_pool.tile([P, D], f32, tag=f"g{k}", name=f"g_ps{k}")
                for k in range(4)
            ]
            if bias:
                r_ps = acc_pool.tile([1, D], f32, tag="r")

            wt_bf = consts.tile([P, 4, E], bf16)    # wt_bf[p, c, s] = W[s, c*128+p]

            for j in range(CH):
                n0, w = starts[j], SIZES[j]
                if j < PRE:
                    h_t, pe_t = h_tiles[j], pe_tiles[j]
                else:
                    h_t = io.tile([P, TPC, D], f32, tag="h")
                    pe_t = io.tile([P, TPC, D], f32, tag="pe")
                    nc.sync.dma_start(h_t[:, 0:w, :], h_re[:, n0:n0 + w, :])
                    nc.scalar.dma_start(pe_t[:, 0:w, :], pe_re[:, n0:n0 + w, :])
                if j == CH - 1:
                    # W^T (bf16) rides the tail of the pe queue (scalar):
                    # consumed only by the final Y matmuls, so it must not
                    # displace h/pe up front; no cast needed.
                    nc.scalar.dma_start(wt_bf[:], wt_re)
                hw_t = io.tile([P, TPC, D], bf16, tag="hw")
                pew_t = io.tile([P, TPC, D], bf16, tag="pew")
                # decay-scale of h on DVE; cast of pe on DVE — NOT on
                # scalar, whose HWDGE queue must keep streaming pe.  For
                # the last two (1-tile) chunks the pe cast moves to ACT,
                # whose DMA issues are done by then, halving the final
                # data->matmul latency chain.
                nc.vector.tensor_tensor(
                    out=hw_t[:, 0:w, :],
                    in0=h_t[:, 0:w, :],
                    in1=wdec_sb[:, n0:n0 + w].unsqueeze(-1).to_broadcast((P, w, D)),
                    op=mybir.AluOpType.mult,
                )
                if j >= CH - 2:
                    nc.scalar.activation(
                        pew_t[:, 0:w, :], pe_t[:, 0:w, :], AF.Copy
                    )
                else:
                    nc.vector.tensor_copy(pew_t[:, 0:w, :], pe_t[:, 0:w, :])
                for i in range(w):
                    n = n0 + i
                    first = n == 0
                    last = n == NT - 1
                    for k in range(4):
                        nc.tensor.matmul(
                            g_ps[k][:],
                            hw_t[:, i, k * P:(k + 1) * P],
                            pew_t[:, i, :],
                            start=first,
                            stop=last,
                        )
                    if bias:
                        nc.tensor.matmul(
                            r_ps[:],
                            wdec_bf[:, n:n + 1],
                            pew_t[:, i, :],
                            start=first,
                            stop=last,
                        )

            # ---- local partial Y = W @ G_shard (+ b outer r_shard) ----
            gred_bf = consts.tile([P, 4, D], bf16)
            for k in range(4):
                eng = nc.vector if k % 2 == 0 else nc.scalar
                if eng is nc.scalar:
                    eng.activation(gred_bf[:, k, :], g_ps[k][:], AF.Copy)
                else:
                    eng.tensor_copy(gred_bf[:, k, :], g_ps[k][:])
            if bias:
                rred_bf = consts.tile([1, D], bf16)
                nc.vector.tensor_copy(rred_bf[:], r_ps[:])

            y_sb = consts.tile([P, 4, D], f32)
            for cs in range(4):
                y_ps = pst.tile([P, D], f32, tag="y")
                for ce in range(4):
                    nc.tensor.matmul(
                        y_ps[:],
                        wt_bf[:, ce, cs * P:(cs + 1) * P],
                        gred_bf[:, ce, :],
                        start=(ce == 0),
                        stop=(not bias and ce == 3),
                    )
                if bias:
                    nc.tensor.matmul(
                        y_ps[:],
                        b_bf[0:1, cs * P:(cs + 1) * P],
                        rred_bf[:],
                        start=False,
                        stop=True,
                    )
                eng = nc.vector if cs % 2 == 0 else nc.scalar
                if eng is nc.scalar:
                    eng.activation(y_sb[:, cs, :], y_ps[:], AF.Copy)
                else:
                    eng.tensor_copy(y_sb[:, cs, :], y_ps[:])
                # per-block output DMA alternating queues: the last store
                # issues as early as possible and receipts overlap
                deng = nc.sync if cs % 2 == 0 else nc.scalar
                deng.dma_start(out_re[:, cs:cs + 1, :], y_sb[:, cs:cs + 1, :])

    nc.compile()
    _GRAPH_CACHE[key] = nc
    return nc


def _in_maps(hidden_states, positional_encodings, W, b):
    w_full = _decay_weights()
    import ml_dtypes

    wt_c = np.ascontiguousarray(
        np.asarray(W, dtype=np.float32).T.astype(ml_dtypes.bfloat16)
    )
    b_c = np.ascontiguousarray(b, dtype=np.float32)
    maps = []
    for c in range(NCORES):
        bi, sh = c // 2, c % 2
        t0, t1 = sh * SS, (sh + 1) * SS
        wdec = np.ascontiguousarray(
            w_full[t0:t1].reshape(NT, P).T, dtype=np.float32
        )
        maps.append(
            {
                "h": np.ascontiguousarray(hidden_states[bi, t0:t1], dtype=np.float32),
                "pe": np.ascontiguousarray(
                    positional_encodings[bi, t0:t1], dtype=np.float32
                ),
                "wt": wt_c,
                "b": b_c,
                "wdec": wdec,
            }
        )
    return maps


def _assemble(results):
    # unshard: the two sequence shards of a batch hold partial decayed
    # states; their sum is the batch's state.
    out = np.empty((B, E, D), dtype=np.float32)
    for bi in range(B):
        np.add(
            results[2 * bi]["out"], results[2 * bi + 1]["out"], out=out[bi]
        )
    return out


def run(hidden_states, positional_encodings, W, b, trace=False, **trace_kwargs):
    from concourse.bass_utils import run_bass_kernel_spmd

    nc = _build(bias=bool(np.any(np.asarray(b) != 0)))
    maps = _in_maps(hidden_states, positional_encodings, W, b)
    res = run_bass_kernel_spmd(
        nc, maps, core_ids=list(range(NCORES)), trace=trace, **trace_kwargs
    )
    return _assemble(res.results), res


def kernel(hidden_states, positional_encodings, W, b):
    out, _ = run(hidden_states, positional_encodings, W, b, trace=False)
    return out
